# revision 9
# baseline (speedup 1.0000x reference)
# Trainium2 Bass kernel for nn_EntityAttentionLayer (sparse entity attention).
#
# Math (per sample b of 8192; a=16 agents, e=32 entities, d=128):
#   q = x@Wq^T, k = x@Wk^T, v = relu(x@Wv^T)
#   s = q k^T/sqrt(d), masked (pre_mask | diag) -> softmax over e -> w
#   out = [x_a, w v] @ Wo^T, rows zeroed where post_mask
#
# Kernel strategy (data parallel over 8 cores, 1024 samples each):
#   - scores via s(i,j) = x_i^T A x_j with A = Wq^T Wk precomputed on host.
#   - host pre-bakes: X^T (bf16, so loads need no DMA transpose), compact
#     post-masked Xa^T (f32 for the precision-critical direct path), and an
#     fp8 additive mask with pre|diag|post all folded in.
#   - per super-block (SB) of 32 samples = 1024 tokens:
#       Za^T = A^T Xa^T               [128, 512]    (PE, const weight)
#       V    = relu(X_hb^T -> WvT)    [tok, 128]x8  (PE + ACT relu)
#       S^T  = mask-mm + X_hb^T Za_hb [128, 8x64]   (PE, psum-accum)
#       P    = exp(S^T/sqrt(d))       (ACT, bf16)
#       csr  = ones^T @ P             (PE; col sums replicated over rows)
#       sc   = exp(-ln(csr+eps))      (ACT twice; avoids DVE's 8x-iterative
#                                      reciprocal which costs 3.4us/SB)
#       P~   = P * sc                 (DVE; pre-normalized softmax weights)
#       att^T= V_hb^T @ P~_hb         [dv, 512]     (PE)
#       out^T= wo1^T Xa^T + wo2^T att^T  (PE, 2 float32r MMs N=512)
#       t_out DVE copy -> DRAM [do, 512]; host transposes back.
#   - post_mask: baked into xa (zeroed cols) and the additive mask (-inf
#     over whole agent cols => P col == 0 => att col == 0).
#   - 4-stage software pipeline so no engine queue ever waits on a
#     same-iteration producer; PSUM pools all fit in 8 banks with bufs=1.
import sys

sys.path.insert(0, "/opt/trn_rl_repo")

import numpy as np
import ml_dtypes

BS, NA, NE, D = 8192, 16, 32, 128
NCORES = 8
S_CORE = BS // NCORES  # 1024 samples per core
SB = 32                # samples per super-block
NSB = S_CORE // SB     # 32 super-blocks per core
HBS = 4                # samples per half-block
NHB = SB // HBS        # 8 half-blocks per SB
TOK = SB * NE          # 1024 tokens per SB
AC = SB * NA           # 512 agent cols per SB
NEG = -57344.0         # fp8e5-representable "minus infinity"
EPS = 1e-6

BF16 = ml_dtypes.bfloat16
FP8 = ml_dtypes.float8_e5m2

_CACHE = {}


def _build():
    import concourse.bacc as bacc
    import concourse.tile as tile
    from concourse import mybir
    from concourse.alu_op_type import AluOpType

    f32 = mybir.dt.float32
    f32r = mybir.dt.float32r
    bf16 = mybir.dt.bfloat16
    fp8 = mybir.dt.float8e5
    ACT = mybir.ActivationFunctionType

    nc = bacc.Bacc("TRN2", target_bir_lowering=False, debug=False,
                   num_devices=NCORES)

    xt = nc.dram_tensor("xt", [D, S_CORE * NE], bf16, kind="ExternalInput")
    xa = nc.dram_tensor("xa", [D, S_CORE * NA], f32r, kind="ExternalInput")
    m8 = nc.dram_tensor("m8", [NSB, 128, NHB * 64], fp8, kind="ExternalInput")
    a_t = nc.dram_tensor("a_t", [D, D], f32r, kind="ExternalInput")
    wvt = nc.dram_tensor("wvt", [D, D], bf16, kind="ExternalInput")
    wo1 = nc.dram_tensor("wo1", [D, D], f32r, kind="ExternalInput")
    wo2 = nc.dram_tensor("wo2", [D, D], f32r, kind="ExternalInput")
    eye8 = nc.dram_tensor("eye8", [128, 128], fp8, kind="ExternalInput")
    out = nc.dram_tensor("out", [NSB, D, AC], f32, kind="ExternalOutput")

    scale = 1.0 / float(np.sqrt(np.float32(D)))

    with tile.TileContext(nc) as tc:
        with (
            tc.tile_pool(name="singles", bufs=1) as singles,
            tc.tile_pool(name="xtp", bufs=3) as xtp,
            tc.tile_pool(name="xap", bufs=5) as xap,
            tc.tile_pool(name="m8p", bufs=3) as m8p,
            tc.tile_pool(name="zap", bufs=2) as zap,
            tc.tile_pool(name="pp", bufs=3) as pp,
            tc.tile_pool(name="pnp", bufs=3) as pnp,
            tc.tile_pool(name="vp", bufs=3) as vp,
            tc.tile_pool(name="scp", bufs=2) as scp,
            tc.tile_pool(name="attnp", bufs=3) as attnp,
            tc.tile_pool(name="outp", bufs=3) as outp,
            tc.tile_pool(name="ps_za", bufs=1, space="PSUM") as ps_za,
            tc.tile_pool(name="ps_s", bufs=1, space="PSUM") as ps_s,
            tc.tile_pool(name="ps_v", bufs=1, space="PSUM") as ps_v,
            tc.tile_pool(name="ps_csr", bufs=1, space="PSUM") as ps_csr,
            tc.tile_pool(name="ps_att", bufs=1, space="PSUM") as ps_att,
            tc.tile_pool(name="ps_out", bufs=2, space="PSUM") as ps_out,
        ):
            s_at = singles.tile([D, D], f32r)
            nc.sync.dma_start(out=s_at, in_=a_t[:, :])
            s_wvt = singles.tile([D, D], bf16)
            nc.sync.dma_start(out=s_wvt, in_=wvt[:, :])
            s_wo1 = singles.tile([D, D], f32r)
            nc.sync.dma_start(out=s_wo1, in_=wo1[:, :])
            s_wo2 = singles.tile([D, D], f32r)
            nc.sync.dma_start(out=s_wo2, in_=wo2[:, :])
            s_eye = singles.tile([128, 128], fp8)
            nc.sync.dma_start(out=s_eye, in_=eye8[:, :])
            s_ones = singles.tile([128, 128], bf16)
            nc.vector.memset(s_ones, 1.0)

            tiles = {}

            def dma_in(g):
                t_xt = xtp.tile([128, TOK], bf16)
                nc.sync.dma_start(out=t_xt, in_=xt[:, g * TOK:(g + 1) * TOK])
                t_xa = xap.tile([128, AC], f32r)
                nc.sync.dma_start(out=t_xa, in_=xa[:, g * AC:(g + 1) * AC])
                t_m8 = m8p.tile([128, NHB * 64], fp8)
                nc.sync.dma_start(out=t_m8, in_=m8[g])
                tiles[g] = dict(xt=t_xt, xa=t_xa, m8=t_m8)

            def front(g):
                tl = tiles[g]
                t_xt, t_xa, t_m8 = tl["xt"], tl["xa"], tl["m8"]
                # Za^T = A^T Xa^T  (f32r matmul: 1 cyc/row at N>=256)
                p_za = ps_za.tile([128, AC], f32)
                nc.tensor.matmul(p_za, s_at, t_xa, start=True, stop=True)
                t_za = zap.tile([128, AC], bf16)
                nc.vector.tensor_copy(t_za, p_za)
                # V token-layout
                p_v = ps_v.tile([128, NHB, D], f32)
                for hb in range(NHB):
                    nc.tensor.matmul(p_v[:, hb, :],
                                     t_xt[:, hb * 128:(hb + 1) * 128],
                                     s_wvt, start=True, stop=True)
                t_v = vp.tile([128, NHB, D], bf16)
                nc.scalar.activation(t_v, p_v, ACT.Relu)
                # S^T psum: mask first, then per-hb score matmuls
                p_s = ps_s.tile([128, NHB * 64], f32)
                nc.tensor.matmul(p_s, s_eye, t_m8, start=True, stop=False,
                                 skip_group_check=True)
                for hb in range(NHB):
                    nc.tensor.matmul(
                        p_s[:, hb * 64:(hb + 1) * 64],
                        t_xt[:, hb * 128:(hb + 1) * 128],
                        t_za[:, hb * 64:(hb + 1) * 64],
                        start=False, stop=(hb == NHB - 1),
                        skip_group_check=True)
                t_p = pp.tile([128, NHB * 64], bf16)
                nc.scalar.activation(t_p, p_s, ACT.Exp, scale=scale)
                tl["v"] = t_v
                tl["p"] = t_p

            def mid1(g):
                tl = tiles[g]
                # csr (replicated col sums), sc = 1/(csr+eps), P~ = P*sc
                p_csr = ps_csr.tile([128, AC], f32)
                nc.tensor.matmul(p_csr, s_ones, tl["p"], start=True, stop=True)
                t_csre = scp.tile([128, AC], f32)
                nc.scalar.activation(t_csre, p_csr, ACT.Copy, bias=EPS)
                t_sc = scp.tile([128, AC], f32)
                nc.vector.reciprocal_approx_fast(out=t_sc, in_=t_csre)
                t_pn = pnp.tile([128, NHB * 64], bf16)
                nc.vector.tensor_tensor(t_pn, tl["p"], t_sc,
                                        op=AluOpType.mult)
                tl["pn"] = t_pn

            def mid2(g):
                tl = tiles[g]
                p_att = ps_att.tile([128, AC], f32)
                t_v, t_pn = tl["v"], tl["pn"]
                for hb in range(NHB):
                    nc.tensor.matmul(p_att[:, hb * 64:(hb + 1) * 64],
                                     t_v[:, hb, :],
                                     t_pn[:, hb * 64:(hb + 1) * 64],
                                     start=True, stop=True,
                                     skip_group_check=True)
                t_attn = attnp.tile([128, AC], f32r)
                nc.scalar.activation(t_attn, p_att, ACT.Copy)
                tl["attn"] = t_attn

            def back(g):
                tl = tiles[g]
                p_o = ps_out.tile([128, AC], f32)
                nc.tensor.matmul(p_o, s_wo1, tl["xa"], start=True, stop=False)
                nc.tensor.matmul(p_o, s_wo2, tl["attn"], start=False,
                                 stop=True)
                t_out = outp.tile([128, AC], f32)
                nc.vector.tensor_copy(t_out, p_o)
                nc.sync.dma_start(out=out[g], in_=t_out)
                del tiles[g]

            dma_in(0)
            for i in range(NSB + 3):
                if i + 1 < NSB:
                    dma_in(i + 1)
                if i < NSB:
                    front(i)
                if 0 <= i - 1 < NSB:
                    mid1(i - 1)
                if 0 <= i - 2 < NSB:
                    mid2(i - 2)
                if 0 <= i - 3 < NSB:
                    back(i - 3)

    nc.compile()
    return nc


def _host_prep(inputs, pre_mask, post_mask, Wq, bq, Wk, bk, Wv, bv, Wo, bo):
    for b in (bq, bk, bv, bo):
        assert not np.any(np.asarray(b)), "kernel assumes zero biases"
    x = np.ascontiguousarray(np.asarray(inputs, np.float32))
    pre = np.asarray(pre_mask)
    post = np.asarray(post_mask)
    Wq = np.asarray(Wq, np.float32)
    Wk = np.asarray(Wk, np.float32)
    Wv = np.asarray(Wv, np.float32)
    Wo = np.asarray(Wo, np.float32)

    a_t = np.ascontiguousarray(Wq.T @ Wk)   # f32 lhsT[c, r] = A[c, r]
    wvt = np.ascontiguousarray(Wv.T).astype(BF16)
    wo1 = np.ascontiguousarray(Wo[:, :D].T)          # f32 [d, do]
    wo2 = np.ascontiguousarray(Wo[:, D:].T)          # f32 [dv, do]
    eye8 = np.eye(128, dtype=FP8)

    # X^T [128, BS*NE] bf16 (pre-transposed on host: plain contiguous DMA)
    x_t = np.ascontiguousarray(x.T.astype(BF16))
    # Xa^T [128, BS*NA] f32, post-mask pre-applied (zero masked agent cols)
    xr = x.reshape(BS, NE, D)
    xa_pm = xr[:, :NA, :] * np.where(post, 0.0, 1.0)[:, :, None]
    xa_t = np.ascontiguousarray(xa_pm.reshape(BS * NA, D).T)

    # masks, blocked layout: per (core, sb): M [128, NHB*64]
    # rows = token-within-hb (32*m + e), cols = 64*hb + 16*m + a
    # pre | diag | post all additive -inf
    pre_all = (pre | np.eye(NE, dtype=bool)[None, :NA, :]
               | post[:, :, None])                          # [BS, A, E]
    m_t = np.where(pre_all, NEG, 0.0).astype(np.float32).transpose(0, 2, 1)
    m_t_g = m_t.reshape(BS // SB, NHB, HBS, NE, NA)  # [g, hb, m, e, a]
    m_comb = np.full((BS // SB, HBS, NE, NHB, HBS, NA), NEG, np.float32)
    for m in range(HBS):
        m_comb[:, m, :, :, m, :] = m_t_g[:, :, m].transpose(0, 2, 1, 3)
    m8 = m_comb.reshape(BS // SB, 128, NHB * 64).astype(FP8)

    per_core = []
    for c in range(NCORES):
        s0 = c * S_CORE
        per_core.append({
            "xt": np.ascontiguousarray(x_t[:, s0 * NE:(s0 + S_CORE) * NE]),
            "xa": np.ascontiguousarray(xa_t[:, s0 * NA:(s0 + S_CORE) * NA]),
            "m8": m8[c * NSB:(c + 1) * NSB],
            "a_t": a_t, "wvt": wvt, "wo1": wo1, "wo2": wo2, "eye8": eye8,
        })
    return per_core


def kernel(inputs, pre_mask, post_mask, Wq, bq, Wk, bk, Wv, bv, Wo, bo,
           _want_results=None):
    from concourse.bass_utils import run_bass_kernel_spmd

    if "nc" not in _CACHE:
        _CACHE["nc"] = _build()
    nc = _CACHE["nc"]

    in_maps = _host_prep(inputs, pre_mask, post_mask, Wq, bq, Wk, bk, Wv, bv,
                         Wo, bo)
    kwargs = dict(_want_results or {})
    res = run_bass_kernel_spmd(nc, in_maps, core_ids=list(range(NCORES)),
                               **kwargs)
    # out per core: [NSB, do, 512] -> [NSB, 512, do] -> [S_CORE*NA, do]
    outs = []
    for r in res.results:
        o = r["out"]
        outs.append(o.transpose(0, 2, 1).reshape(S_CORE * NA, D))
    out = np.concatenate(outs, axis=0)
    if _want_results is not None:
        _CACHE["last_results"] = res
    return out.reshape(BS, NA, D)
